# revision 14
# baseline (speedup 1.0000x reference)
"""MLA (multi-headed latent attention) forward on 8 Trainium2 NeuronCores.

Sharding: data-parallel over batch (4) x tensor-parallel over heads (2):
core c handles batch c//2 with heads [16*(c%2), 16*(c%2)+16).
Each core computes a partial (H-dim) output contribution; host sums the
TP pair and stacks batches.

All matmuls run in bf16 (fp32 PSUM accumulation). Weights are host-
pre-tiled into the exact SBUF layouts so every DMA is contiguous.
LN means are folded into the fused projection as extra weight columns;
only the sum-of-squares statistics need dedicated ones-matmuls.
Layout is feature-major ("T" = [feature, token]) everywhere except v
(token-major for the PV matmul).
"""

import numpy as np
import ml_dtypes
import concourse.bass as bass
import concourse.mybir as mybir
import concourse.tile as tile
from concourse import bacc
from concourse import bass_utils

F32 = mybir.dt.float32
BF16 = mybir.dt.bfloat16
AX = mybir.AxisListType
OP = mybir.AluOpType
AF = mybir.ActivationFunctionType
NPBF = ml_dtypes.bfloat16

B, S, H, NH = 4, 1024, 4096, 32
QL, KVL, RD, ND, VD = 1536, 512, 64, 128, 128
QHD = ND + RD  # 192
EPS = 1e-6
NCORES = 8
TP = 2                 # tensor-parallel ways (heads)
HPC = NH // TP         # 16 heads per core
G = 2                  # heads per group
NG = HPC // G          # 8 groups
TOKT = S // 128        # 8 token tiles
KH = H // 128          # 32 contraction tiles for H
NMT = 17               # phase-1 m-tiles: 1 pe+stats, 4 kv, 12 qa
SCALE = float(QHD) ** -0.5

# rope feature permutation: pairs (d, d+32) land 16 lanes apart within a
# 32-partition quadrant so stream_shuffle can do rotate_half.
DIMS_PERM = np.array(
    list(range(0, 16)) + list(range(32, 48))
    + list(range(16, 32)) + list(range(48, 64)), dtype=np.int64)
SHUF_MASK = [(i + 16) % 32 for i in range(32)]

_NC_CACHE = {}


def _build_nc():
    nc = bacc.Bacc("TRN2", target_bir_lowering=False, debug=False)

    hs_d = nc.dram_tensor("hs_d", (128, KH, S), BF16, kind="ExternalInput").ap()
    wa_d = nc.dram_tensor("wa_d", (128, NMT, KH, 128), BF16, kind="ExternalInput").ap()
    wqb_d = nc.dram_tensor("wqb_d", (128, NG, 3, 12, 128), BF16, kind="ExternalInput").ap()
    wk_d = nc.dram_tensor("wk_d", (128, NG, 2, 4, 128), BF16, kind="ExternalInput").ap()
    wv_d = nc.dram_tensor("wv_d", (128, NG, 4, 256), BF16, kind="ExternalInput").ap()
    wo_d = nc.dram_tensor("wo_d", (128, KH, HPC, 128), BF16, kind="ExternalInput").ap()
    csq_d = nc.dram_tensor("csq_d", (128, S), BF16, kind="ExternalInput").ap()
    ssq_d = nc.dram_tensor("ssq_d", (128, S), BF16, kind="ExternalInput").ap()
    tri_d = nc.dram_tensor("tri_d", (128, 128), BF16, kind="ExternalInput").ap()
    ones_d = nc.dram_tensor("ones_d", (128, 1), BF16, kind="ExternalInput").ap()
    outT = nc.dram_tensor("outT", (H, S), F32, kind="ExternalOutput").ap()

    with tile.TileContext(nc) as tc:
        with tc.tile_pool(name="pers", bufs=1) as pers:
            # ---------------- persistent tiles ----------------
            q_anT = pers.tile([128, 12 * S], BF16)     # LN(q_a)^T  (1536, 1024)
            kv_cnT = pers.tile([128, 4 * S], BF16)     # LN(kv_c)^T (512, 1024)
            kpeT2 = pers.tile([128, S], BF16)          # roped k_pe^T, both halves
            csq_t = pers.tile([128, S], BF16)
            ssq_t = pers.tile([128, S], BF16)
            tri_t = pers.tile([128, 128], BF16)
            ones_t = pers.tile([128, 1], BF16)
            nc.sync.dma_start(out=csq_t[:, :], in_=csq_d)
            nc.sync.dma_start(out=ssq_t[:, :], in_=ssq_d)
            nc.sync.dma_start(out=tri_t[:, :], in_=tri_d)
            nc.sync.dma_start(out=ones_t[:, :], in_=ones_d)

            # ======== phase 1 (feature-major): X^T = Wa^T @ hs^T, LN fused ====
            # m-tile order: pe+stats first, then kv, then qa — so the kv path
            # (and k_pe rope) is finished early and phase-2 kv work can start
            # while the qa LN is still being applied.
            M_TILES = [("pe", 0)] + [("kv", i) for i in range(4)] \
                + [("qa", i) for i in range(12)]

            with tc.tile_pool(name="hsp", bufs=1) as hsp, \
                 tc.tile_pool(name="p1wa", bufs=3) as p1wa, \
                 tc.tile_pool(name="sqp", bufs=2) as sqp, \
                 tc.tile_pool(name="rowp", bufs=1) as rowp, \
                 tc.tile_pool(name="bcp", bufs=2) as bcp, \
                 tc.tile_pool(name="p1ps", bufs=3, space="PSUM") as p1ps, \
                 tc.tile_pool(name="stps", bufs=1, space="PSUM") as stps:
                hst = [hsp.tile([128, 4, S], BF16, name=f"hst_{i}")
                       for i in range(8)]
                for i in range(8):
                    nc.scalar.dma_start(
                        out=hst[i][:, :, :],
                        in_=hs_d[:, 4 * i:4 * (i + 1), :])

                # sum-of-squares stats accumulators (fp32 PSUM)
                stat = {("qa", 0): stps.tile([1, 512], F32, name="st_qa0"),
                        ("qa", 1): stps.tile([1, 512], F32, name="st_qa1"),
                        ("kv", 0): stps.tile([1, 512], F32, name="st_kv0"),
                        ("kv", 1): stps.tile([1, 512], F32, name="st_kv1")}
                mrow_qa = rowp.tile([1, S], F32, name="mrow_qa")
                mrow_kv = rowp.tile([1, S], F32, name="mrow_kv")

                def ln_apply(kind):
                    nmt, n_feat, destT = (
                        (4, KVL, kv_cnT) if kind == "kv" else (12, QL, q_anT))
                    mr = mrow_kv if kind == "kv" else mrow_qa
                    mb = bcp.tile([128, S], F32, tag="bc", name=f"mb_{kind}")
                    rb = bcp.tile([128, S], F32, tag="bc2", name=f"rb_{kind}")
                    sq_row = rowp.tile([1, S], F32, tag="sqr", name=f"sqr_{kind}")
                    for qh in range(2):
                        sl = slice(qh * 512, qh * 512 + 512)
                        nc.vector.tensor_scalar_mul(
                            sq_row[:, sl], stat[(kind, qh)][:, :], 1.0 / n_feat)
                    nc.gpsimd.partition_broadcast(mb[:, :], mr[:, :])
                    nc.gpsimd.partition_broadcast(rb[:, :], sq_row[:, :])
                    # var = E[x^2] - mean^2 ; rstd = 1/sqrt(var + eps)
                    tmp = bcp.tile([128, S], F32, tag="tmp", name=f"tmp_{kind}")
                    nc.vector.tensor_tensor(out=tmp[:, :], in0=mb[:, :],
                                            in1=mb[:, :], op=OP.mult)
                    nc.vector.tensor_tensor(out=rb[:, :], in0=rb[:, :],
                                            in1=tmp[:, :], op=OP.subtract)
                    nc.vector.tensor_scalar_add(rb[:, :], rb[:, :], EPS)
                    nc.scalar.activation(rb[:, :], rb[:, :], AF.Sqrt)
                    nc.vector.reciprocal_approx_fast(out=rb[:, :], in_=rb[:, :])
                    # 16-bit copies of the broadcast rows: all-bf16 applies get
                    # 2x DVE throughput, and the work splits across two engines
                    mb16 = bcp.tile([128, S], BF16, tag="bc16", name=f"mb16_{kind}")
                    rb16 = bcp.tile([128, S], BF16, tag="bc216", name=f"rb16_{kind}")
                    nc.vector.tensor_scalar_mul(mb16[:, :], mb[:, :], 1.0)
                    nc.scalar.copy(rb16[:, :], rb[:, :])
                    for mi in range(nmt):
                        dsl = destT[:, mi * S:(mi + 1) * S]
                        nc.vector.tensor_tensor(out=dsl, in0=dsl, in1=mb16[:, :],
                                                op=OP.subtract)
                        nc.vector.tensor_tensor(out=dsl, in0=dsl, in1=rb16[:, :],
                                                op=OP.mult)

                for (kind, mi) in M_TILES:
                    tix = {"pe": 0, "kv": 1, "qa": 5}[kind] + mi
                    wt = p1wa.tile([128, KH, 128], BF16, tag="wa",
                                   name=f"wa_{kind}_{mi}")
                    nc.sync.dma_start(out=wt[:, :, :], in_=wa_d[:, tix, :, :])
                    if kind == "qa":
                        dest = q_anT[:, mi * S:(mi + 1) * S]
                    elif kind == "kv":
                        dest = kv_cnT[:, mi * S:(mi + 1) * S]
                    else:
                        dest = kpeT2[0:64, :]
                    for qh in range(2):
                        sl = slice(qh * 512, qh * 512 + 512)
                        ps = p1ps.tile([128, 512], F32, tag="p1")
                        for k in range(KH):
                            nc.tensor.matmul(
                                ps[:, :], wt[:, k, :], hst[k // 4][:, k % 4, sl],
                                start=(k == 0), stop=(k == KH - 1))
                        if kind == "pe":
                            nc.scalar.copy(dest[:, sl], ps[0:64, :])
                            # rows 64/96 carry sum(qa), sum(kv) over features
                            nc.vector.tensor_scalar_mul(
                                mrow_qa[:, sl], ps[64:65, :], 1.0 / QL)
                            nc.vector.tensor_scalar_mul(
                                mrow_kv[:, sl], ps[96:97, :], 1.0 / KVL)
                        elif qh == 0:
                            nc.scalar.copy(dest[:, sl], ps[:, :])
                        else:
                            nc.vector.tensor_scalar_mul(dest[:, sl], ps[:, :], 1.0)
                    if kind != "pe":
                        # accumulate sum(x^2) for this tile into the stats psum
                        nmt = 12 if kind == "qa" else 4
                        sqt = sqp.tile([128, S], BF16, tag="sq")
                        nc.scalar.activation(sqt[:, :], dest, AF.Square)
                        for qh in range(2):
                            sl = slice(qh * 512, qh * 512 + 512)
                            nc.tensor.matmul(
                                stat[(kind, qh)][:, :], ones_t[:, :], sqt[:, sl],
                                start=(mi == 0), stop=(mi == nmt - 1))
                        if kind == "kv" and mi == 3:
                            ln_apply("kv")

                # ---- rope k_pe on rows 0:64, then duplicate into 64:128 ----
                kp_sh = sqp.tile([64, S], BF16, tag="kpsh", name="kpsh")
                nc.vector.stream_shuffle(
                    kp_sh[:, :].bitcast(F32), kpeT2[0:64, :].bitcast(F32),
                    SHUF_MASK)
                nc.vector.tensor_tensor(out=kp_sh[:, :], in0=kp_sh[:, :],
                                        in1=ssq_t[:64, :], op=OP.mult)
                nc.vector.tensor_tensor(out=kpeT2[0:64, :], in0=kpeT2[0:64, :],
                                        in1=csq_t[:64, :], op=OP.mult)
                nc.vector.tensor_tensor(out=kpeT2[0:64, :], in0=kpeT2[0:64, :],
                                        in1=kp_sh[:, :], op=OP.add)
                nc.sync.dma_start(out=kpeT2[64:128, :], in_=kpeT2[0:64, :])

                ln_apply("qa")

            # ======== phase 2 + 3 ========
            with tc.tile_pool(name="otp", bufs=1) as otp:
                oT = otp.tile([128, HPC * S], BF16)    # normalized o^T
                with tc.tile_pool(name="gq2", bufs=2) as gqp, \
                     tc.tile_pool(name="wop", bufs=4) as wop, \
                     tc.tile_pool(name="op", bufs=2) as outp, \
                     tc.tile_pool(name="gkv", bufs=4) as gkvp, \
                     tc.tile_pool(name="wq", bufs=3) as wqp, \
                     tc.tile_pool(name="wk", bufs=3) as wkp, \
                     tc.tile_pool(name="wv", bufs=2) as wvp, \
                     tc.tile_pool(name="rshp", bufs=2) as rshp, \
                     tc.tile_pool(name="pp", bufs=6) as ppool, \
                     tc.tile_pool(name="denp", bufs=4) as denp, \
                     tc.tile_pool(name="pjps", bufs=2, space="PSUM") as pjps, \
                     tc.tile_pool(name="sps", bufs=2, space="PSUM") as sps, \
                     tc.tile_pool(name="ops", bufs=2, space="PSUM") as ops, \
                     tc.tile_pool(name="smps", bufs=2, space="PSUM") as smps:
                    def emit_knv(g):
                        # k_nope^T (2 m-tiles) and v (token-major): these only
                        # depend on the kv path, so they are emitted ahead of
                        # the group that consumes them to keep the PE fed while
                        # the qa LN / previous attention finishes.
                        knT = gkvp.tile([128, 2 * S], BF16, tag="knT",
                                        name=f"knT_{g}")
                        for m in range(2):
                            wt = wkp.tile([128, 4, 128], BF16, tag="wk",
                                          name=f"wk_{g}_{m}")
                            nc.sync.dma_start(out=wt[:, :, :],
                                              in_=wk_d[:, g, m, :, :])
                            for qh in range(2):
                                ps = pjps.tile([128, 512], F32, tag="pj")
                                for k in range(4):
                                    nc.tensor.matmul(
                                        ps[:, :], wt[:, k, :],
                                        kv_cnT[:, k * S + qh * 512:
                                               k * S + qh * 512 + 512],
                                        start=(k == 0), stop=(k == 3))
                                nc.scalar.copy(knT[:, m * S + qh * 512:
                                                   m * S + qh * 512 + 512],
                                               ps[:, :])

                        v_sb = gkvp.tile([128, TOKT * G * VD], BF16, tag="v",
                                         name=f"v_{g}")
                        wv_t = wvp.tile([128, 4, 256], BF16, tag="wv",
                                        name=f"wv_{g}")
                        nc.sync.dma_start(out=wv_t[:, :, :], in_=wv_d[:, g, :, :])
                        for t in range(TOKT):
                            ps = pjps.tile([128, 512], F32, tag="pj")
                            for k in range(4):
                                nc.tensor.matmul(
                                    ps[:, :256],
                                    kv_cnT[:, k * S + t * 128:
                                           k * S + (t + 1) * 128],
                                    wv_t[:, k, :], start=(k == 0), stop=(k == 3))
                            nc.vector.tensor_scalar_mul(
                                v_sb[:, t * 256:(t + 1) * 256], ps[:, :256], 1.0)
                        return knT, v_sb

                    knv = {}
                    for gg in range(4):
                        knv[gg] = emit_knv(gg)
                    for g in range(NG):
                        knT, v_sb = knv.pop(g)

                        # ---- q^T for this group: 2x nope m-tiles + 1 pe pair
                        qT = gqp.tile([128, 3 * S], BF16, tag="qT")
                        for m in range(3):
                            wt = wqp.tile([128, 12, 128], BF16, tag="wqb",
                                          name=f"wqb_{g}_{m}")
                            nc.sync.dma_start(out=wt[:, :, :],
                                              in_=wqb_d[:, g, m, :, :])
                            for qh in range(2):
                                sl = slice(qh * 512, qh * 512 + 512)
                                ps = pjps.tile([128, 512], F32, tag="pj")
                                for k in range(12):
                                    nc.tensor.matmul(
                                        ps[:, :], wt[:, k, :],
                                        q_anT[:, k * S + qh * 512:
                                              k * S + qh * 512 + 512],
                                        start=(k == 0), stop=(k == 11))
                                nc.scalar.copy(
                                    qT[:, m * S + qh * 512:
                                       m * S + qh * 512 + 512], ps[:, :])
                        # rope the pe tile (m=2): rows 0:64 head0, 64:128 head1
                        pe = qT[:, 2 * S:3 * S]
                        rsh = rshp.tile([128, S], BF16, tag="rsh")
                        nc.vector.stream_shuffle(
                            rsh[:, :].bitcast(F32), pe.bitcast(F32), SHUF_MASK)
                        nc.vector.tensor_tensor(out=rsh[:, :], in0=rsh[:, :],
                                                in1=ssq_t[:, :], op=OP.mult)
                        nc.vector.tensor_tensor(out=pe, in0=pe,
                                                in1=csq_t[:, :], op=OP.mult)
                        nc.vector.tensor_tensor(out=pe, in0=pe, in1=rsh[:, :],
                                                op=OP.add)

                        # ---- attention per head, qh-outer ----
                        for hh in range(G):
                            hg = g * G + hh
                            for qh in range(2):
                                nik = 4 * (qh + 1)
                                po = ops.tile([128, 512], F32, tag="po")
                                psm = smps.tile([1, 512], F32, tag="psm")
                                for ik in range(nik):
                                    lo = max(128 * ik, 512 * qh)
                                    hi = 512 * (qh + 1)
                                    w = hi - lo
                                    ps_s = sps.tile([128, 512], F32, tag="ps")
                                    nc.tensor.matmul(
                                        ps_s[:, :w],
                                        knT[:, hh * S + ik * 128:
                                            hh * S + (ik + 1) * 128],
                                        qT[:, hh * S + lo: hh * S + hi],
                                        start=True, stop=False)
                                    nc.tensor.matmul(
                                        ps_s[:, :w],
                                        kpeT2[hh * 64:(hh + 1) * 64,
                                              ik * 128:(ik + 1) * 128],
                                        qT[hh * 64:(hh + 1) * 64,
                                           2 * S + lo: 2 * S + hi],
                                        start=False, stop=True)
                                    p = ppool.tile([128, 512], BF16, tag="p")
                                    nc.scalar.activation(p[:, :w], ps_s[:, :w],
                                                         AF.Exp, scale=SCALE)
                                    if lo == 128 * ik:
                                        nc.vector.tensor_tensor(
                                            out=p[:, 0:128], in0=p[:, 0:128],
                                            in1=tri_t[:, :], op=OP.mult)
                                    nc.tensor.matmul(
                                        psm[:, lo - 512 * qh: hi - 512 * qh],
                                        ones_t[:, :], p[:, :w],
                                        start=(ik == 0), stop=(ik == nik - 1))
                                    nc.tensor.matmul(
                                        po[:, lo - 512 * qh: hi - 512 * qh],
                                        v_sb[:, ik * 256 + hh * 128:
                                             ik * 256 + (hh + 1) * 128],
                                        p[:, :w],
                                        start=(ik == 0), stop=(ik == nik - 1))
                                # denominator: row -> sbuf -> bcast -> recip
                                srow = denp.tile([1, 512], F32, tag="dr")
                                nc.scalar.copy(srow[:, :], psm[:, :])
                                rbc = denp.tile([128, 512], F32, tag="db")
                                nc.gpsimd.partition_broadcast(rbc[:, :],
                                                              srow[:, :])
                                nc.vector.reciprocal_approx_fast(
                                    out=rbc[:, :], in_=rbc[:, :])
                                nc.vector.tensor_tensor(
                                    out=oT[:, hg * S + qh * 512:
                                           hg * S + qh * 512 + 512],
                                    in0=po[:, :], in1=rbc[:, :], op=OP.mult)
                        if g + 4 < NG:
                            knv[g + 4] = emit_knv(g + 4)

                    # ======== phase 3: out^T = Wo^T @ o ========
                    if True:
                        wops = sps  # reuse the scores PSUM pool (same shape)
                        for hr in range(KH):
                            wt = wop.tile([128, HPC, 128], BF16, tag="wo",
                                          name=f"wo_{hr}")
                            nc.sync.dma_start(out=wt[:, :, :], in_=wo_d[:, hr, :, :])
                            ot = outp.tile([128, S], F32, tag="out")
                            for qh in range(2):
                                sl = slice(qh * 512, qh * 512 + 512)
                                ps = wops.tile([128, 512], F32, tag="ps")
                                for m in range(HPC):
                                    nc.tensor.matmul(
                                        ps[:, :], wt[:, m, :],
                                        oT[:, m * S + qh * 512:
                                           m * S + qh * 512 + 512],
                                        start=(m == 0), stop=(m == HPC - 1))
                                if qh == 0:
                                    nc.scalar.copy(ot[:, sl], ps[:, :])
                                else:
                                    nc.vector.tensor_scalar_mul(ot[:, sl], ps[:, :], 1.0)
                            nc.sync.dma_start(
                                out=outT[hr * 128:(hr + 1) * 128, :], in_=ot[:, :])
    nc.compile()
    return nc


def _host_prep(inputs):
    hs = np.asarray(inputs["hidden_states"], np.float32)
    cos = np.asarray(inputs["cos"], np.float32)
    sin = np.asarray(inputs["sin"], np.float32)
    pid = np.asarray(inputs["position_ids"]).astype(np.int64)
    Wqa = np.asarray(inputs["Wqa"], np.float32)
    gqa = np.asarray(inputs["gqa"], np.float32)
    Wqb = np.asarray(inputs["Wqb"], np.float32)
    Wkva = np.asarray(inputs["Wkva"], np.float32)
    gkva = np.asarray(inputs["gkva"], np.float32)
    Wkvb = np.asarray(inputs["Wkvb"], np.float32)
    Wo = np.asarray(inputs["Wo"], np.float32)

    # phase-1 fused projection: [pe'+sums | kv x4 | qa x12] m-tiles.
    # pe tile cols: 0:64 rope-permuted Wkva-pe, 64 sum(Wqa cols),
    # 65 sum(Wkva kv cols) — yields feature-sum rows for the LN means.
    wsum_qa = Wqa.sum(axis=1, keepdims=True)
    wsum_kv = Wkva[:, :KVL].sum(axis=1, keepdims=True)
    pe_cols = np.concatenate(
        [Wkva[:, KVL:][:, DIMS_PERM], wsum_qa, np.zeros((H, 31), np.float32),
         wsum_kv, np.zeros((H, 31), np.float32)], axis=1)
    wa = np.concatenate([pe_cols, Wkva[:, :KVL], Wqa], axis=1)  # (H, 2176)
    # -> (128, 17, 32, 128): [p, mtile, k, col]
    wa_t = np.ascontiguousarray(
        wa.reshape(KH, 128, NMT, 128).transpose(1, 2, 0, 3)).astype(NPBF)

    # fold LN gains into the B-projections (bias terms are zero per spec)
    Wqb = Wqb * gqa[:, None]
    Wkvb = Wkvb * gkva[:, None]

    # sign pattern for the shuffle-based rotate_half
    sign = np.where(DIMS_PERM < RD // 2, -1.0, 1.0).astype(np.float32)[:, None]

    tri = np.zeros((128, 128), np.float32)
    kp, q = np.mgrid[0:128, 0:128]
    tri[q >= kp] = 1.0

    per_core = []
    w4 = Wqb.reshape(QL, NH, QHD)
    wk4 = Wkvb.reshape(KVL, NH, ND + VD)
    for c in range(NCORES):
        b, t = divmod(c, TP)
        heads = slice(t * HPC, (t + 1) * HPC)
        # Wqb: group-blocked [h0 nope | h1 nope | h0 pe' | h1 pe'] per group
        wq = w4[:, heads]                       # (QL, 16, 192)
        nope = wq[:, :, :ND]                    # (QL, 16, 128)
        pe = wq[:, :, ND:][:, :, DIMS_PERM]     # (QL, 16, 64) permuted
        blocks = []
        for g in range(NG):
            blocks.extend([nope[:, 2 * g], nope[:, 2 * g + 1],
                           pe[:, 2 * g], pe[:, 2 * g + 1]])
        wqb_c = np.concatenate(blocks, axis=1)  # (QL, NG*384)
        # -> (128, NG, 3, 12, 128)
        wqb_t = np.ascontiguousarray(
            wqb_c.reshape(12, 128, NG, 3, 128).transpose(1, 2, 3, 0, 4)
        ).astype(NPBF)

        wkc = wk4[:, heads]
        wkvbk_c = wkc[:, :, :ND].reshape(KVL, HPC * ND)
        # -> (128, NG, 2, 4, 128)
        wk_t = np.ascontiguousarray(
            wkvbk_c.reshape(4, 128, NG, 2, 128).transpose(1, 2, 3, 0, 4)
        ).astype(NPBF)
        wkvbv_c = wkc[:, :, ND:].reshape(KVL, HPC * VD)
        # -> (128, NG, 4, 256)
        wv_t = np.ascontiguousarray(
            wkvbv_c.reshape(4, 128, NG, 256).transpose(1, 2, 0, 3)
        ).astype(NPBF)

        wo_c = Wo[t * HPC * VD:(t + 1) * HPC * VD]   # (2048, 4096)
        # -> (128, 32, 16, 128)
        wo_t = np.ascontiguousarray(
            wo_c.reshape(HPC, 128, KH, 128).transpose(1, 2, 0, 3)).astype(NPBF)

        cos_g = cos[pid[b]]                     # (S, RD)
        sin_g = sin[pid[b]]
        cosT = cos_g.T[DIMS_PERM]               # (64, S)
        sinT = sin_g.T[DIMS_PERM]
        csq = np.ascontiguousarray(np.vstack([cosT, cosT])).astype(NPBF)
        ssq = np.ascontiguousarray(np.vstack([sinT * sign, sinT * sign])).astype(NPBF)

        hsT = hs[b].T                           # (H, S)
        hs_t = np.ascontiguousarray(
            hsT.reshape(KH, 128, S).transpose(1, 0, 2)).astype(NPBF)

        per_core.append({
            "hs_d": hs_t,
            "wa_d": wa_t,
            "wqb_d": wqb_t,
            "wk_d": wk_t,
            "wv_d": wv_t,
            "wo_d": wo_t,
            "csq_d": csq,
            "ssq_d": ssq,
            "tri_d": tri.astype(NPBF),
            "ones_d": np.ones((128, 1), NPBF),
        })
    return per_core


def kernel(**inputs):
    if "nc" not in _NC_CACHE:
        _NC_CACHE["nc"] = _build_nc()
    nc = _NC_CACHE["nc"]
    in_maps = _host_prep(inputs)
    res = bass_utils.run_bass_kernel_spmd(nc, in_maps, core_ids=list(range(NCORES)))
    outs = []
    for b in range(B):
        acc = res.results[TP * b]["outT"].astype(np.float32)
        for t in range(1, TP):
            acc = acc + res.results[TP * b + t]["outT"]
        outs.append(acc.T)
    return np.stack(outs, axis=0)
